# revision 25
# baseline (speedup 1.0000x reference)
"""Bidirectional temporal attention on 8 Trainium2 NeuronCores.

Problem: x[1,16,256,768] -> per-head QKV projection (12 heads, hd=64),
heads 0-5 causal ("lookback"), heads 6-11 anti-causal ("lookahead"),
softmax over keys, concat heads, output projection.

Sharding: queries are strided-interleaved across the 8 cores
(core c owns queries q with q % 8 == c).  This makes the program
SPMD-uniform: every core runs the identical instruction stream; all
core-dependence lives in the input data (its x columns and its mask
tables).  K/V are computed sharded (core c projects sequence rows
[512c, 512c+512)) and shared with one AllGather (~1.5MB bf16 per core).

On-chip layout: scores are computed transposed (S^T[k, q]) so the
softmax denominator comes for free from a ones-column appended to V
(PV matmul accumulates sum(exp) in row 64), and exp() runs on the
scalar engine straight out of PSUM.  exp() uses no max-subtraction:
score scale for this problem is ~N(0, 0.3), |s| < 4, so exp is safe
(verified against the reference in test.py).
"""
import os
import sys

sys.path.insert(0, "/opt/trn_rl_repo")

import numpy as np
import ml_dtypes

import concourse.bass as bass
import concourse.bacc as bacc
import concourse.tile as tile
from concourse import mybir
from concourse.bass_utils import run_bass_kernel_spmd

BF16 = ml_dtypes.bfloat16

S = 4096          # sequence length (16*256)
D = 768           # model dim
H = 12            # heads
HD = 64           # head dim
NLB = 6           # lookback heads
NC = 8            # cores
QC = S // NC      # queries per core (512)
CH = D // 128     # contraction chunks (6)
KT_N = S // 128   # k-tiles (32)
SCALE = 1.0 / 8.0 # 1/sqrt(hd)

_BUILT = None
LAST_RESULT = None

# --- custom DVE exp: out = (c0 + c1*s + c2*s^2)^8 ~= exp(s/8) -------------
# deg-2 minimax on s/64 in [-0.3, 0.3] then 3 squarings; worst-case rel
# err ~0.9% over |s|<=19 (scores here have |s|<~16).
_EXPC = (1.0002497254795546, 0.015799323897119764, 0.00012138390752253904)
_EXP4_OP = None


def _register_exp_op():
    global _EXP4_OP
    if _EXP4_OP is not None:
        return _EXP4_OP
    from concourse import dve_ops
    from concourse.dve_spec import Spec, Src0, C0, C1, C2, sq, lower
    from concourse.dve_uop import DveOpSpec

    name = "EXP8_ANT"
    spec = Spec(
        body=sq(sq(sq(C0 + Src0 * C1 + sq(Src0) * C2))),
        reference=lambda in0, in1, c0, c1, c2:
            (c0 + c1 * in0 + c2 * in0 * in0) ** 8,
    )
    row = max(dve_ops._SUB_OPCODE_FOR_NAME.values()) + 1
    dve_ops._SUB_OPCODE_FOR_NAME[name] = row
    shas = {}
    for ver in ("v3", "v4"):
        uops = lower(spec, ver=ver)
        shas[ver] = DveOpSpec(name=name, opcode=row, uops=uops,
                              rd1_en=False).sha(ver)
    op = dve_ops.DveOp(name, spec, subdim=False, uops_sha=shas)
    dve_ops.OPS.append(op)
    dve_ops.CUSTOM_DVE_SPECS[name] = spec
    _EXP4_OP = op
    return op





# Persistent NEFF cache: compile_bir_kernel is content-pure (BIR json ->
# neff bytes), so cache across processes/directories keyed by sha256.
_NEFF_CACHE_DIR = os.path.expanduser("~/.cache/bass_neff_cache")


def _install_neff_cache():
    import hashlib
    import shutil
    from concourse import bass_utils, bass2jax

    if getattr(bass_utils.compile_bir_kernel, "_cached_wrapper", False):
        return
    orig = bass_utils.compile_bir_kernel

    def cached(bir_json, tmpdir, neff_name="file.neff"):
        try:
            os.makedirs(_NEFF_CACHE_DIR, exist_ok=True)
            key = hashlib.sha256(
                bir_json if isinstance(bir_json, bytes)
                else bir_json.encode()).hexdigest()
            path = os.path.join(_NEFF_CACHE_DIR, key + ".neff")
            out_path = os.path.join(tmpdir, neff_name)
            if os.path.exists(path):
                shutil.copyfile(path, out_path)
                return out_path
            res = orig(bir_json, tmpdir, neff_name)
            shutil.copyfile(res, path)
            return res
        except Exception:
            return orig(bir_json, tmpdir, neff_name)

    cached._cached_wrapper = True
    bass_utils.compile_bir_kernel = cached
    bass2jax.compile_bir_kernel = cached


def _build(sim=False, repeat=1):
    """Build + compile the SPMD program (identical on all 8 cores).

    sim=True replaces the collective with a local DMA so the single-core
    cost-model simulator (TimelineSim) can run; timing-only, data garbage.
    """
    exp_op = _register_exp_op()
    nc = bacc.Bacc("TRN2", target_bir_lowering=False, debug=False,
                   num_devices=NC)
    f32, bf16 = mybir.dt.float32, mybir.dt.bfloat16

    xq_in = nc.dram_tensor("xq", [D, QC], bf16, kind="ExternalInput")
    xkv_in = nc.dram_tensor("xkv", [D, QC], bf16, kind="ExternalInput")
    wq_in = nc.dram_tensor("wq", [D, D], bf16, kind="ExternalInput")
    wk_in = nc.dram_tensor("wk", [D, D], bf16, kind="ExternalInput")
    wv_in = nc.dram_tensor("wv", [D, D], bf16, kind="ExternalInput")
    wo_in = nc.dram_tensor("wo", [D, D], bf16, kind="ExternalInput")
    bq_in = nc.dram_tensor("bq", [D], f32, kind="ExternalInput")
    bk_in = nc.dram_tensor("bk", [D], f32, kind="ExternalInput")
    bv_in = nc.dram_tensor("bv", [D], f32, kind="ExternalInput")
    bo_in = nc.dram_tensor("bo", [D], f32, kind="ExternalInput")
    mk_in = nc.dram_tensor("masks", [128, 16, 128], bf16, kind="ExternalInput")
    out_ext = nc.dram_tensor("out", [QC, D], bf16, kind="ExternalOutput")

    # AllGather bounce buffers (separate KT and V gathers) bf16.
    KSZ = D * QC
    agin_k = nc.dram_tensor("agin_k", [KSZ], bf16)
    agout_k = nc.dram_tensor("agout_k", [NC * KSZ], bf16, addr_space="Shared")
    agin_v = nc.dram_tensor("agin_v", [KSZ], bf16)
    agout_v = nc.dram_tensor("agout_v", [NC * KSZ], bf16, addr_space="Shared")

    def kt_region(base_ap, chunk=None):
        off = 0 if chunk is None else chunk * KSZ
        return base_ap[off:off + KSZ].rearrange("(p a b) -> p a b", a=128, b=QC)

    def v_region(base_ap, chunk=None):
        off = 0 if chunk is None else chunk * KSZ
        return base_ap[off:off + KSZ].rearrange("(s a b) -> s a b", a=128, b=D)

    # weight loads as single DMAs: dram [768, 768] -> sbuf [128, 6, 768]
    def w_view(src):
        return src[:, :].rearrange("(c p) n -> p c n", p=128)

    with tile.TileContext(nc) as tc:
        with (
            tc.tile_pool(name="persist", bufs=1) as persist,
            tc.tile_pool(name="stage", bufs=1) as stage,
        ):
            projin_cm = tc.tile_pool(name="projin", bufs=1)
            projin = projin_cm.__enter__()
            # ---- KV-critical loads first (SP + ACT queues) ----------
            xkv_sb = projin.tile([128, CH, QC], bf16, tag="xkv")
            nc.sync.dma_start(
                out=xkv_sb,
                in_=xkv_in[:, :].rearrange("(c p) n -> p c n", p=128))
            wk_sb = projin.tile([128, CH, D], bf16, tag="wk")
            nc.scalar.dma_start(out=wk_sb, in_=w_view(wk_in))
            wv_sb = projin.tile([128, CH, D], bf16, tag="wv")
            nc.sync.dma_start(out=wv_sb, in_=w_view(wv_in))
            bk_sb = projin.tile([128, CH], f32, tag="bk")
            nc.scalar.dma_start(
                out=bk_sb, in_=bk_in[:].rearrange("(a b) -> b a", b=128))
            bv_bc = persist.tile([128, D], f32, tag="bv")
            sap = bv_in[:]
            nc.scalar.dma_start(out=bv_bc, in_=bass.AP(
                tensor=sap.tensor, offset=sap.offset, ap=[[0, 128]] + sap.ap))

            # ---- phase A1: K/V projections -> bounce ----------------
            with tc.tile_pool(name="pj_ps", bufs=2, space="PSUM") as pj_ps:
                kt_st = stage.tile([128, CH, QC], bf16, tag="ktst")
                for p in range(CH):
                    ps = pj_ps.tile([128, QC], f32, tag="pjq")
                    cols = slice(128 * p, 128 * p + 128)
                    for d in range(CH):
                        nc.tensor.matmul(ps, wk_sb[:, d, cols], xkv_sb[:, d, :],
                                         start=(d == 0), stop=(d == CH - 1))
                    nc.vector.tensor_scalar_add(kt_st[:, p, :], ps,
                                                bk_sb[:, p:p + 1])
                nc.sync.dma_start(
                    out=kt_region(agin_k[:]).rearrange("p a b -> a p b"),
                    in_=kt_st)
                if sim:
                    for i in range(NC):
                        nc.sync.dma_start(
                            out=agout_k[i * KSZ:i * KSZ + 64],
                            in_=agin_k[0:64])
                else:
                    nc.gpsimd.collective_compute(
                        "AllGather", mybir.AluOpType.bypass,
                        replica_groups=[list(range(NC))],
                        ins=[agin_k[:].opt()], outs=[agout_k[:].opt()])

                v_st = stage.tile([128, 4, D], bf16, tag="vst")
                for s4 in range(4):
                    rows = slice(128 * s4, 128 * s4 + 128)
                    psa = pj_ps.tile([128, 512], f32, tag="pjva")
                    psb = pj_ps.tile([128, 256], f32, tag="pjvb")
                    for d in range(CH):
                        lt = xkv_sb[:, d, rows]
                        nc.tensor.matmul(psa, lt, wv_sb[:, d, 0:512],
                                         start=(d == 0), stop=(d == CH - 1))
                        nc.tensor.matmul(psb, lt, wv_sb[:, d, 512:768],
                                         start=(d == 0), stop=(d == CH - 1))
                    # transpose (h,e)->(e,h) on the write so the gathered
                    # chunk unpacks contiguously; PV lhsT reads stride-12
                    va = v_st[:, s4, :].rearrange("p (e h) -> p h e", h=H)
                    nc.vector.tensor_add(va[:, 0:8, :], psa.rearrange(
                        "p (h e) -> p h e", e=HD), bv_bc[:, 0:512].rearrange(
                        "p (h e) -> p h e", e=HD))
                    nc.vector.tensor_add(va[:, 8:12, :], psb.rearrange(
                        "p (h e) -> p h e", e=HD), bv_bc[:, 512:768].rearrange(
                        "p (h e) -> p h e", e=HD))
                nc.sync.dma_start(
                    out=v_region(agin_v[:]).rearrange("s a b -> a s b"),
                    in_=v_st)

            # ---- AllGather V ----------------------------------------
            if sim:
                for i in range(NC):
                    nc.sync.dma_start(out=agout_v[i * KSZ:i * KSZ + 64],
                                      in_=agin_v[0:64])
            else:
                nc.gpsimd.collective_compute(
                    "AllGather", mybir.AluOpType.bypass,
                    replica_groups=[list(range(NC))],
                    ins=[agin_v[:].opt()], outs=[agout_v[:].opt()])

            # ---- Q-side loads (overlap gather); masks before wo/bo,
            # which are not needed until the output projection ----------
            mask_sb = persist.tile([128, 16, 128], bf16, tag="masks")
            nc.scalar.dma_start(out=mask_sb, in_=mk_in[:, :, :])
            xq_sb = projin.tile([128, CH, QC], bf16, tag="xq")
            nc.sync.dma_start(
                out=xq_sb, in_=xq_in[:, :].rearrange("(c p) n -> p c n", p=128))
            wq_sb = projin.tile([128, CH, D], bf16, tag="wq")
            nc.scalar.dma_start(out=wq_sb, in_=w_view(wq_in))
            bq_sb = projin.tile([128, CH], f32, tag="bq")
            nc.scalar.dma_start(
                out=bq_sb, in_=bq_in[:].rearrange("(a b) -> b a", b=128))
            # pre-warm the ACT Exp table so the first score tile doesn't
            # pay the function-set load
            warm = persist.tile([1, 2], f32, tag="actwarm")
            nc.vector.memset(warm, 0.0)
            nc.scalar.activation(out=warm, in_=warm,
                                 func=mybir.ActivationFunctionType.Exp)
            wo_sb = persist.tile([128, CH, D], bf16, tag="wo")
            nc.scalar.dma_start(out=wo_sb, in_=w_view(wo_in))
            bo_bc = persist.tile([128, D], f32, tag="bo")
            sap = bo_in[:]
            nc.scalar.dma_start(out=bo_bc, in_=bass.AP(
                tensor=sap.tensor, offset=sap.offset, ap=[[0, 128]] + sap.ap))

            # ---- Q projection (overlaps gather) ---------------------
            with tc.tile_pool(name="pq_ps", bufs=2, space="PSUM") as pq_ps:
                qt_sb = persist.tile([128, CH, QC], bf16, tag="qt")
                for p in range(CH):
                    ps = pq_ps.tile([128, QC], f32, tag="pqq")
                    cols = slice(128 * p, 128 * p + 128)
                    for d in range(CH):
                        nc.tensor.matmul(ps, wq_sb[:, d, cols], xq_sb[:, d, :],
                                         start=(d == 0), stop=(d == CH - 1))
                    nc.vector.tensor_scalar_add(qt_sb[:, p, :], ps,
                                                bq_sb[:, p:p + 1])

            projin_cm.__exit__(None, None, None)
            pt_cm = tc.tile_pool(name="pt_pool", bufs=4)
            pt_pool = pt_cm.__enter__()
            norm_cm = tc.tile_pool(name="norm", bufs=2)
            norm_pool = norm_cm.__enter__()

            # ---- unpack K^T and V' (chunks from both ends) ----------
            # kt_c[i]: [128(=2 heads x 64), pair, 512 seq] per gather chunk
            kt_c, v_c = [None] * NC, [None] * NC
            for i in (0, 7, 1, 6, 2, 5, 3, 4):
                kt_c[i] = persist.tile([128, CH, QC], bf16, tag=f"ktc{i}",
                                       name=f"ktc{i}")
                nc.sync.dma_start(
                    out=kt_c[i],
                    in_=kt_region(agout_k[:], i).rearrange("p a b -> a p b"))
                v_c[i] = persist.tile([128, 4, (HD + 1) * H], bf16,
                                      tag=f"vc{i}", name=f"vc{i}")
                nc.vector.memset(v_c[i][:, :, HD * H:], 1.0)
                nc.scalar.dma_start(
                    out=v_c[i][:, :, 0:HD * H],
                    in_=v_region(agout_v[:], i).rearrange("s a b -> a s b"))

            # ---- phase B: attention (head pairs, row-group packed) --
            ot_sb = persist.tile([128, CH, QC], bf16, tag="ot")
            attn_ps = tc.tile_pool(name="attn_ps", bufs=3, space="PSUM")
            sc_ps = attn_ps.__enter__()
            pvpool = tc.tile_pool(name="pv_ps", bufs=2, space="PSUM")
            pv_ps = pvpool.__enter__()
            for _rep, pr in [(r, p) for r in range(repeat)
                             for p in (0, 3, 1, 4, 2, 5)]:
                lb = pr < 3
                rows = (slice(0, 64), slice(64, 128))
                pv2 = [pv_ps.tile([65, QC], f32, tag="pv",
                                  name=f"pv{_rep}_{pr}{ab}")
                       for ab in range(2)]
                for b in range(4):      # blocks of 8 k-tiles
                    cols = slice(128 * b, QC) if lb else slice(0, QC - 128 * b)
                    pt2 = [pt_pool.tile([128, 8, QC], bf16, tag="pt",
                                        name=f"pt{_rep}_{pr}{b}{ab}")
                           for ab in range(2)]
                    for gg in range(4):  # score groups of 2 k-tiles
                        sc2 = [sc_ps.tile([128, 2, QC], f32, tag="sc",
                                          name=f"sc{_rep}_{pr}_{b}_{gg}_{ab}") for ab in range(2)]
                        for t in range(2):
                            mm = 8 * b + 2 * gg + t
                            ktn = mm if lb else KT_N - 1 - mm
                            kk = slice(128 * (ktn % 4), 128 * (ktn % 4) + 128)
                            for ab in range(2):
                                # ab=1 runs in array rows 64-127, concurrent
                                nc.tensor.matmul(
                                    sc2[ab][:, t, cols],
                                    kt_c[ktn // 4][rows[ab], pr, kk],
                                    qt_sb[rows[ab], pr, cols],
                                    start=True, stop=True)
                        for ab in range(2):
                            # stagger the DVE-exp slot across the two packed
                            # heads (A: first group, B: last) so ACT and DVE
                            # both stay fed at every gg step
                            if (gg, ab) in ((0, 0), (3, 1)):
                                nc.vector._custom_dve(
                                    exp_op,
                                    out=pt2[ab][:, 2 * gg:2 * gg + 2, cols],
                                    in0=sc2[ab][:, :, cols],
                                    s0=_EXPC[0], s1=_EXPC[1], imm2=_EXPC[2])
                            else:
                                nc.scalar.activation(
                                    out=pt2[ab][:, 2 * gg:2 * gg + 2, cols],
                                    in_=sc2[ab][:, :, cols],
                                    func=mybir.ActivationFunctionType.Exp,
                                    scale=SCALE)
                    # mask the partial (diagonal) query tile of this block
                    mq = slice(128 * b, 128 * b + 128) if lb else \
                        slice(QC - 128 * (b + 1), QC - 128 * b)
                    moff = 0 if lb else 8
                    for ab in range(2):
                        nc.vector.tensor_mul(
                            pt2[ab][:, :, mq], pt2[ab][:, :, mq],
                            mask_sb[:, moff:moff + 8, :])
                    # PV for this block (denominator rides in row 64)
                    for mm8 in range(8):
                        mm = 8 * b + mm8
                        ktn = mm if lb else KT_N - 1 - mm
                        vck = v_c[ktn // 4][:, ktn % 4, :].rearrange(
                            "p (e h) -> p h e", h=H)
                        for ab in range(2):
                            nc.tensor.matmul(
                                pv2[ab][:, cols], vck[:, 2 * pr + ab, :],
                                pt2[ab][:, mm8, cols],
                                start=(mm == 0), stop=(mm == KT_N - 1))
                # normalize: rows 0-63 / row 64, into the out-proj operand
                for ab in range(2):
                    rc = norm_pool.tile([1, QC], f32, tag="rc")
                    nc.vector.reciprocal(rc, pv2[ab][64:65, :])
                    rb = norm_pool.tile([64, QC], f32, tag="rb")
                    nc.gpsimd.partition_broadcast(rb, rc)
                    nc.vector.tensor_mul(ot_sb[rows[ab], pr, :],
                                         pv2[ab][0:64, :], rb)
            pvpool.__exit__(None, None, None)
            attn_ps.__exit__(None, None, None)

            norm_cm.__exit__(None, None, None)
            pt_cm.__exit__(None, None, None)

            # ---- phase C: output projection -------------------------
            with tc.tile_pool(name="op_ps", bufs=2, space="PSUM") as op_ps:
                ob = stage.tile([128, 4, D], bf16, tag="ob")
                for qb in range(4 * repeat):
                    qb = qb % 4
                    qcols = slice(128 * qb, 128 * qb + 128)
                    psa = op_ps.tile([128, 512], f32, tag="opa")
                    psb = op_ps.tile([128, 256], f32, tag="opb")
                    for pch in range(CH):
                        lt = ot_sb[:, pch, qcols]
                        nc.tensor.matmul(psa, lt, wo_sb[:, pch, 0:512],
                                         start=(pch == 0), stop=(pch == CH - 1))
                        nc.tensor.matmul(psb, lt, wo_sb[:, pch, 512:768],
                                         start=(pch == 0), stop=(pch == CH - 1))
                    nc.vector.tensor_add(ob[:, qb, 0:512], psa, bo_bc[:, 0:512])
                    nc.vector.tensor_add(ob[:, qb, 512:768], psb,
                                         bo_bc[:, 512:768])
                nc.sync.dma_start(
                    out=out_ext[:, :].rearrange("(q p) n -> p q n", p=128),
                    in_=ob)

    nc.compile()
    return nc


def _host_prep(x, Wq, bq, Wk, bk, Wv, bv, Wo, bo):
    xT = np.ascontiguousarray(
        x.reshape(S, D).T).astype(BF16)          # [768, 4096]
    wq_t = np.ascontiguousarray(
        Wq.transpose(1, 0, 2).reshape(D, D)).astype(BF16)
    wk_t = np.ascontiguousarray(
        Wk.transpose(1, 0, 2).reshape(D, D)).astype(BF16)
    wv_t = np.ascontiguousarray(
        Wv.transpose(1, 0, 2).reshape(D, D)).astype(BF16)
    wo_m = np.ascontiguousarray(Wo).astype(BF16)
    common = {
        "wq": wq_t, "wk": wk_t, "wv": wv_t, "wo": wo_m,
        "bq": bq.reshape(D).astype(np.float32),
        "bk": bk.reshape(D).astype(np.float32),
        "bv": bv.reshape(D).astype(np.float32),
        "bo": bo.reshape(D).astype(np.float32),
    }
    k_idx = np.arange(128)[:, None]
    n_idx = np.arange(128)[None, :]
    in_maps = []
    for c in range(NC):
        masks = np.zeros((128, 16, 128), dtype=BF16)
        for m in range(8):
            masks[:, m, :] = (128 * m + k_idx <= 8 * n_idx + c)
            masks[:, 8 + m, :] = (128 * (7 - m) + k_idx >= 8 * n_idx + c)
        in_maps.append({
            **common,
            "xq": np.ascontiguousarray(xT[:, c::NC]),
            "xkv": np.ascontiguousarray(xT[:, QC * c:QC * (c + 1)]),
            "masks": masks,
        })
    return in_maps




# ---------------------------------------------------------------------------
# Cached PJRT runner: same semantics as bass2jax.run_bass_via_pjrt for the
# 8-core SPMD case, but the jitted executable is built once and reused, so
# repeat kernel() calls skip retracing (~1.6s/call -> ~transfer+exec).
_RUNNER = None


def _make_runner(nc):
    import jax
    from jax.sharding import Mesh, PartitionSpec
    from jax.experimental.shard_map import shard_map
    from concourse import bass2jax, mybir as _mb

    bass2jax.install_neuronx_cc_hook()
    partition_name = (nc.partition_id_tensor.name
                      if nc.partition_id_tensor else None)
    in_names, out_names, out_avals, zero_shapes = [], [], [], []
    for alloc in nc.m.functions[0].allocations:
        if not isinstance(alloc, _mb.MemoryLocationSet):
            continue
        name = alloc.memorylocations[0].name
        if alloc.kind == "ExternalInput":
            if name != partition_name:
                in_names.append(name)
        elif alloc.kind == "ExternalOutput":
            shape = tuple(alloc.tensor_shape)
            dtype = _mb.dt.np(alloc.dtype)
            out_names.append(name)
            out_avals.append(jax.core.ShapedArray(shape, dtype))
            zero_shapes.append((shape, dtype))
    n_params = len(in_names)
    all_names = in_names + out_names
    if partition_name is not None:
        all_names = all_names + [partition_name]
    donate = tuple(range(n_params, n_params + len(out_names)))

    def _body(*args):
        operands = list(args)
        if partition_name is not None:
            operands.append(bass2jax.partition_id_tensor())
        outs = bass2jax._bass_exec_p.bind(
            *operands,
            out_avals=tuple(out_avals),
            in_names=tuple(all_names),
            out_names=tuple(out_names),
            lowering_input_output_aliases=(),
            sim_require_finite=True,
            sim_require_nnan=True,
            nc=nc,
        )
        return tuple(outs)

    devices = jax.devices()[:NC]
    mesh = Mesh(np.asarray(devices), ("core",))
    in_specs = (PartitionSpec("core"),) * (n_params + len(out_names))
    out_specs = (PartitionSpec("core"),) * len(out_names)
    sharded = jax.jit(
        shard_map(_body, mesh=mesh, in_specs=in_specs, out_specs=out_specs,
                  check_rep=False),
        donate_argnums=donate, keep_unused=True)

    from jax.sharding import NamedSharding
    import jax.numpy as jnp
    shard = NamedSharding(mesh, PartitionSpec("core"))
    static_names = {"wq", "wk", "wv", "wo", "bq", "bk", "bv", "bo", "masks"}
    static_cache = {}

    def _zeros():
        return tuple(jnp.zeros((NC * s[0], *s[1:]), d) for s, d in zero_shapes)
    zeros_fn = jax.jit(_zeros, out_shardings=(shard,) * len(zero_shapes))

    import hashlib

    def run(in_maps):
        concat_in = []
        for nm in in_names:
            if nm in static_names:
                host = np.concatenate([np.asarray(in_maps[c][nm])
                                       for c in range(NC)], axis=0)
                key = hashlib.sha1(host.tobytes()).hexdigest()
                cached = static_cache.get(nm)
                if cached is None or cached[0] != key:
                    cached = (key, jax.device_put(host, shard))
                    static_cache[nm] = cached
                concat_in.append(cached[1])
            else:
                concat_in.append(np.concatenate(
                    [np.asarray(in_maps[c][nm]) for c in range(NC)], axis=0))
        out_arrs = sharded(*concat_in, *zeros_fn())
        return [
            {nm: np.asarray(out_arrs[i]).reshape(NC, *out_avals[i].shape)[c]
             for i, nm in enumerate(out_names)}
            for c in range(NC)
        ]

    return run


def kernel(x, Wq, bq, Wk, bk, Wv, bv, Wo, bo):
    global _BUILT, _RUNNER
    args = [np.asarray(a, dtype=np.float32)
            for a in (x, Wq, bq, Wk, bk, Wv, bv, Wo, bo)]
    if _BUILT is None:
        _install_neff_cache()
        _BUILT = _build()
        _RUNNER = _make_runner(_BUILT)
    in_maps = _host_prep(*args)
    results = _RUNNER(in_maps)
    out_full = np.empty((S, D), dtype=np.float32)
    for c in range(NC):
        out_full[c::NC] = results[c]["out"].astype(np.float32)
    return out_full.reshape(1, 16, 256, D)
